# revision 2
# baseline (speedup 1.0000x reference)
"""Trainium2 Bass kernel for BlockAttnResLayer.

Computation (reference):
  V = concat([blocks, partial[None]])            # [9, B*T, D]
  rms = sqrt(mean(V^2, -1) + 1e-8)
  logits[n,t] = (V[n,t,:] . (norm_scale*proj_w)) / rms[n,t]
  alpha = softmax(logits, axis=n)
  h = sum_n alpha * V
  f = gelu(h @ W1) @ W2                          # tanh-approx gelu
  new_partial = partial + f
  returns (h, new_partial)

Sharding: pure data-parallel over tokens (B*T = 4096 -> 512/core on 8 cores).
Weights replicated; FFN matmuls run in float32r (fp32 with 11-bit mantissa,
1 cycle/row on the PE at N>=256 vs 4 cycles/row for plain fp32).

Overlap structure (overlap=True): attention emits token-tiles 0,1 first;
MM1 then runs over token-half A (N=256) for every f-chunk while attention
finishes tiles 2,3 on DVE/ACT, spilling gelu(half-a) to a DRAM scratch.
After attention, MM1 half-B re-streams W1 and MM2 consumes half-B act from
SBUF plus half-A act read back from DRAM, accumulating F quarter-groups
into an SBUF accumulator.  Costs ~96 MiB extra DMA, buys ~A/2 of PE overlap.
"""
import numpy as np
from contextlib import ExitStack

import concourse.bass as bass
import concourse.bacc as bacc
import concourse.tile as tile
from concourse import mybir
from concourse.bass_utils import run_bass_kernel_spmd
from concourse.masks import make_identity

f32 = mybir.dt.float32
f32r = mybir.dt.float32r
AF = mybir.ActivationFunctionType
ALU = mybir.AluOpType

N_CORES = 8
NB = 8            # completed blocks
N1 = 9            # blocks + partial
B, T, D, F = 2, 2048, 2048, 8192
TOK = B * T       # 4096
TPC = TOK // N_CORES  # 512 tokens per core
P = 128
TT = TPC // P     # 4 token tiles per core
TH = TPC // 2     # 256-token halves
DC = D // P       # 16 d-chunks
FC = F // P       # 64 f-chunks
NG = 4            # f-chunk quarter groups for MM2 accumulation
FG = FC // NG     # 16 f-chunks per group
NQ = D // 512     # 4 output column quarters
EPS = 1e-8


def round_f32r(x: np.ndarray) -> np.ndarray:
    """RNE-round fp32 to 11 explicit mantissa bits (the PE's fp32r format)."""
    v = x.astype(np.float32).view(np.uint32).astype(np.uint64)
    lsb = (v >> 12) & 1
    v = v + 0x7FF + lsb
    v = (v & np.uint64(0xFFFFF000)).astype(np.uint32)
    return v.view(np.float32)


def retile_w1(w1r: np.ndarray) -> np.ndarray:
    """[D, F] -> [FC, P, DC, P] with w1t[fc, p, kc, q] = W1[kc*P+p, fc*P+q]."""
    return np.ascontiguousarray(
        w1r.reshape(DC, P, FC, P).transpose(2, 1, 0, 3))


def build_nc(n_reps: int = 1, gelu: bool = True, phase_a: bool = True,
             phase_b: bool = True, overlap: bool = True):
    act_fn = AF.Gelu_apprx_tanh if gelu else AF.Copy
    nc = bacc.Bacc("TRN2", target_bir_lowering=False, debug=False, num_devices=N_CORES)
    vb = nc.dram_tensor("vb", [N1, TPC, D], f32, kind="ExternalInput").ap()
    # w1 host-retiled to [FC, P, DC, P]: w1t[fc, p, kc, q] = W1[kc*128+p, fc*128+q]
    # so each weight-tile DMA reads one contiguous 8KB run per partition.
    w1 = nc.dram_tensor("w1", [FC, P, DC, P], f32r, kind="ExternalInput").ap()
    w2 = nc.dram_tensor("w2", [F, D], f32r, kind="ExternalInput").ap()
    pjw = nc.dram_tensor("pjw", [D], f32, kind="ExternalInput").ap()
    nsw = nc.dram_tensor("nsw", [D], f32, kind="ExternalInput").ap()
    h_out = nc.dram_tensor("h_out", [TPC, D], f32, kind="ExternalOutput").ap()
    np_out = nc.dram_tensor("np_out", [TPC, D], f32, kind="ExternalOutput").ap()
    act_d = nc.dram_tensor("act_d", [FC, P, TH], f32r).ap()   # half-A act spill

    h_out_t = h_out.rearrange("(tt p) d -> tt p d", p=P)

    with tile.TileContext(nc) as tc, ExitStack() as ctx:
        outer = ctx.enter_context(tc.tile_pool(name="outer", bufs=1))
        pw_b = outer.tile([P, D], f32)
        # transposed h in two token-halves: hTs[half][k] is [P, TH]
        hTs = [[outer.tile([P, TH], f32r, name=f"hT{hf}_{k}") for k in range(DC)]
               for hf in range(2)]
        # FFN pools that must be live during attention for overlap
        w1p = ctx.enter_context(tc.tile_pool(name="w1p", bufs=2))
        ps1p = ctx.enter_context(tc.tile_pool(name="ps1p", bufs=2, space="PSUM"))
        aspp = ctx.enter_context(tc.tile_pool(name="aspp", bufs=6))

        def mm1_half(hf, fc, dst_ap, w1t=None):
            """One f-chunk of MM1 over token half hf -> gelu -> dst_ap (SBUF).

            Weight DMAs ride the ACT engine's HW queue so they never head-of-line
            block the V-tile loads on the SP queue."""
            if w1t is None:
                w1t = w1p.tile([P, DC, P], f32r, name="w1t")
                nc.scalar.dma_start(out=w1t, in_=w1[fc])
            ps1 = ps1p.tile([P, TH], f32, name="ps1")
            for k in range(DC):
                nc.tensor.matmul(ps1[:], lhsT=w1t[:, k, :], rhs=hTs[hf][k][:],
                                 start=(k == 0), stop=(k == DC - 1))
            nc.scalar.activation(dst_ap, ps1[:], act_fn)
            return w1t

        for _rep in range(n_reps):
            # ---------------- Phase A: block attention -> h, hT ----------------
            if not phase_a:
                zp = ctx.enter_context(tc.tile_pool(name="zp", bufs=1))
                zt = zp.tile([P, TH], f32)
                nc.vector.memset(zt, 0.001)
                for hf in range(2):
                    for k in range(DC):
                        nc.scalar.activation(hTs[hf][k][:], zt[:], AF.Copy)
            if phase_a:
              with ExitStack() as ctxA:
                vpool = ctxA.enter_context(tc.tile_pool(name="vpool", bufs=9))
                spool = ctxA.enter_context(tc.tile_pool(name="spool", bufs=1))
                sqps = ctxA.enter_context(tc.tile_pool(name="sqps", bufs=1, space="PSUM"))
                small = ctxA.enter_context(tc.tile_pool(name="small", bufs=3))
                hpool = ctxA.enter_context(tc.tile_pool(name="hpool", bufs=2))
                psumT = ctxA.enter_context(tc.tile_pool(name="psumT", bufs=2, space="PSUM"))
                consts = ctxA.enter_context(tc.tile_pool(name="consts", bufs=1))

                ident = consts.tile([P, P], f32)
                make_identity(nc, ident)
                eps_t = consts.tile([P, 1], f32)
                nc.vector.memset(eps_t, EPS)
                nb_t = spool.tile([P, D], f32, name="dsc")
                nsw_b = bass.AP(tensor=nsw.tensor, offset=nsw.offset,
                                ap=[[0, P], *nsw.ap])
                nc.gpsimd.dma_start(out=nb_t, in_=nsw_b)
                pj_t = hpool.tile([P, D], f32, name="ht")
                pjw_b = bass.AP(tensor=pjw.tensor, offset=pjw.offset,
                                ap=[[0, P], *pjw.ap])
                nc.gpsimd.dma_start(out=pj_t, in_=pjw_b)
                nc.vector.tensor_mul(pw_b[:], nb_t[:], pj_t[:])

                def attn_tile(tt):
                    ss9 = small.tile([P, N1], f32, name="ss9")
                    dp9 = small.tile([P, N1], f32, name="dp9")
                    vts = []
                    for n in range(N1):
                        v = vpool.tile([P, D], f32, name="vt")
                        nc.sync.dma_start(out=v, in_=vb[n, tt * P:(tt + 1) * P, :])
                        vts.append(v)
                        sq = sqps.tile([P, D], f32, name="sq")
                        nc.scalar.activation(sq[:], v[:], AF.Square,
                                             accum_out=ss9[:, n:n + 1])
                        dsc = spool.tile([P, D], f32, name="dsc")
                        nc.vector.scalar_tensor_tensor(
                            out=dsc[:], in0=v[:], scalar=1.0, in1=pw_b[:],
                            op0=ALU.mult, op1=ALU.mult, accum_out=dp9[:, n:n + 1])
                    rms9 = small.tile([P, N1], f32, name="rms9")
                    nc.scalar.activation(rms9[:], ss9[:], AF.Sqrt,
                                         bias=eps_t[:], scale=1.0 / D)
                    inv9 = small.tile([P, N1], f32, name="inv9")
                    nc.vector.reciprocal(inv9[:], rms9[:])
                    lg9 = small.tile([P, N1], f32, name="lg9")
                    nc.vector.tensor_mul(lg9[:], dp9[:], inv9[:])
                    mx1 = small.tile([P, 1], f32, name="mx1")
                    nc.vector.tensor_reduce(mx1[:], lg9[:], axis=mybir.AxisListType.X,
                                            op=ALU.max)
                    nc.vector.tensor_scalar_sub(lg9[:], lg9[:], mx1[:])
                    e9 = small.tile([P, N1], f32, name="e9")
                    se1 = small.tile([P, 1], f32, name="se1")
                    nc.scalar.activation(e9[:], lg9[:], AF.Exp, accum_out=se1[:])
                    invs = small.tile([P, 1], f32, name="invs")
                    nc.vector.reciprocal(invs[:], se1[:])
                    al9 = small.tile([P, N1], f32, name="al9")
                    nc.vector.tensor_scalar_mul(al9[:], e9[:], invs[:])

                    h_t = hpool.tile([P, D], f32, name="ht")
                    nc.vector.tensor_scalar_mul(h_t[:], vts[0][:], al9[:, 0:1])
                    for n in range(1, N1):
                        nc.vector.scalar_tensor_tensor(
                            out=h_t[:], in0=vts[n][:], scalar=al9[:, n:n + 1],
                            in1=h_t[:], op0=ALU.mult, op1=ALU.add)
                    nc.sync.dma_start(out=h_out_t[tt], in_=h_t[:])
                    hf, col = divmod(tt, 2)
                    for k in range(DC):
                        pst = psumT.tile([P, P], f32, name="pst")
                        nc.tensor.transpose(pst[:], h_t[:, k * P:(k + 1) * P], ident[:])
                        nc.scalar.activation(
                            hTs[hf][k][:, col * P:(col + 1) * P], pst[:], AF.Copy)

                attn_tile(0)
                attn_tile(1)
                if phase_b and overlap:
                    # MM1 over token-half A for every f-chunk, spilled to DRAM,
                    # overlapping attention tiles 2,3 on DVE/ACT.
                    for fc in range(FC):
                        a_sb = aspp.tile([P, TH], f32r, name="asp")
                        mm1_half(0, fc, a_sb[:])
                        nc.scalar.dma_start(out=act_d[fc], in_=a_sb[:])
                attn_tile(2)
                attn_tile(3)

            # ---------------- Phase B: FFN (f32r) + residual ----------------
            if phase_b:
              with ExitStack() as ctxB:
                w2p = ctxB.enter_context(tc.tile_pool(name="w2p", bufs=4))
                actap = ctxB.enter_context(tc.tile_pool(name="actap", bufs=FG + 4))
                actbp = ctxB.enter_context(tc.tile_pool(name="actbp", bufs=FG + 4))
                oap = ctxB.enter_context(tc.tile_pool(name="oap", bufs=1))
                evp = ctxB.enter_context(tc.tile_pool(name="evp", bufs=4))
                ptp = ctxB.enter_context(tc.tile_pool(name="ptp", bufs=4))
                ps2p = ctxB.enter_context(tc.tile_pool(name="ps2p", bufs=4, space="PSUM"))

                out_acc = [oap.tile([P, D], f32, name=f"oa{m}") for m in range(TT)]

                for g in range(NG):
                    act_a, act_b = [], []
                    for fcl in range(FG):
                        gfc = g * FG + fcl
                        if overlap:
                            # readback half-A act, compute half-B act
                            aa = actap.tile([P, TH], f32r, name="acta")
                            nc.sync.dma_start(out=aa, in_=act_d[gfc])
                            ab = actbp.tile([P, TH], f32r, name="actb")
                            mm1_half(1, gfc, ab[:])
                        else:
                            aa = actap.tile([P, TH], f32r, name="acta")
                            ab = actbp.tile([P, TH], f32r, name="actb")
                            w1t = mm1_half(0, gfc, aa[:])
                            mm1_half(1, gfc, ab[:], w1t=w1t)
                        act_a.append(aa)
                        act_b.append(ab)

                    for q in range(NQ):
                        ps2 = [ps2p.tile([P, 512], f32, name="ps2") for _ in range(TT)]
                        for fcl in range(FG):
                            gfc = g * FG + fcl
                            w2t = w2p.tile([P, 512], f32r, name="w2t")
                            nc.sync.dma_start(
                                out=w2t,
                                in_=w2[gfc * P:(gfc + 1) * P, q * 512:(q + 1) * 512])
                            for m in range(TT):
                                src = act_a[fcl] if m < 2 else act_b[fcl]
                                nc.tensor.matmul(
                                    ps2[m][:],
                                    lhsT=src[:, (m % 2) * P:(m % 2 + 1) * P],
                                    rhs=w2t[:],
                                    start=(fcl == 0), stop=(fcl == FG - 1))
                        for m in range(TT):
                            if g == 0:
                                nc.vector.tensor_copy(
                                    out_acc[m][:, q * 512:(q + 1) * 512], ps2[m][:])
                            elif g < NG - 1:
                                nc.vector.tensor_add(
                                    out_acc[m][:, q * 512:(q + 1) * 512], ps2[m][:],
                                    out_acc[m][:, q * 512:(q + 1) * 512])
                            else:
                                ev = evp.tile([P, 512], f32, name="ev")
                                nc.vector.tensor_add(
                                    ev[:], ps2[m][:],
                                    out_acc[m][:, q * 512:(q + 1) * 512])
                                pt = ptp.tile([P, 512], f32, name="pt")
                                nc.sync.dma_start(
                                    out=pt,
                                    in_=vb[NB, m * P:(m + 1) * P, q * 512:(q + 1) * 512])
                                nc.vector.tensor_add(ev[:], ev[:], pt[:])
                                nc.sync.dma_start(
                                    out=np_out[m * P:(m + 1) * P, q * 512:(q + 1) * 512],
                                    in_=ev[:])

    nc.compile()
    return nc


_NC = None


def _get_nc():
    global _NC
    if _NC is None:
        _NC = build_nc()
    return _NC


def make_in_maps(inputs):
    blocks = np.ascontiguousarray(np.asarray(inputs["blocks"], dtype=np.float32)).reshape(NB, TOK, D)
    pb = np.ascontiguousarray(np.asarray(inputs["partial_block"], dtype=np.float32)).reshape(TOK, D)
    w1r = retile_w1(round_f32r(np.asarray(inputs["ffn_w1"], dtype=np.float32)))
    w2r = round_f32r(np.asarray(inputs["ffn_w2"], dtype=np.float32))
    pjw = np.ascontiguousarray(np.asarray(inputs["proj_w"], dtype=np.float32))
    nsw = np.ascontiguousarray(np.asarray(inputs["norm_scale"], dtype=np.float32))

    in_maps = []
    for c in range(N_CORES):
        sl = slice(c * TPC, (c + 1) * TPC)
        vbc = np.concatenate([blocks[:, sl], pb[None, sl]], axis=0)
        in_maps.append({"vb": vbc, "w1": w1r, "w2": w2r, "pjw": pjw, "nsw": nsw})
    return in_maps


def kernel(blocks, partial_block, proj_w, norm_scale, ffn_w1, ffn_w2):
    in_maps = make_in_maps(dict(blocks=blocks, partial_block=partial_block,
                                proj_w=proj_w, norm_scale=norm_scale,
                                ffn_w1=ffn_w1, ffn_w2=ffn_w2))
    nc = _get_nc()
    res = run_bass_kernel_spmd(nc, in_maps, list(range(N_CORES)))
    h = np.concatenate([r["h_out"] for r in res.results], axis=0).reshape(B, T, D)
    npar = np.concatenate([r["np_out"] for r in res.results], axis=0).reshape(B, T, D)
    return h, npar

